# revision 1
# baseline (speedup 1.0000x reference)
"""Trainium2 Bass kernel for nn_Attention_66563403153646.

Dense transformer attention block with rotary embeddings + gated adapter
(prefix) attention, fp32 reference:

    y = softmax(rope(x@wq) @ rope(x@wk).T * k + mask) @ (x@wv)
      + gate * softmax(rope(x@wq) @ (adapter@wk).T * k) @ (adapter@wv)
    out = y @ wo

Sharding: 4-way tensor-parallel over heads x 2-way data-parallel over batch
(8 NeuronCores). Each core computes a [S, D] partial of its batch's output
(its 8 heads' contribution through wo); the host sums the 4 TP partials.

All matmul operands are bf16 (PE 1 cycle/row); accumulation is fp32 in
PSUM. q/k/v and the attention output stay resident in SBUF for the whole
kernel -- no DRAM scratch round trips. Layouts:
  - x is fed transposed ([D, S]) so projections contract D on partitions;
    it streams through SBUF in 512-column tiles while per-head weight
    tiles are (re)loaded per x-tile.
  - q/k live per-head in [HD, S] layout with rope-pair-permuted head dims
    (host permutes wq/wk columns: even rope dims first, odd second) so
    RoPE is two partition-halves of elementwise ops writing straight into
    the persistent SBUF tiles.
  - scores are computed per 512-col q tile over 128-row k blocks
    (causal: only blocks <= diagonal; the diagonal block gets a 0/1
    triangle mask).
  - p blocks land in [k, q] layout, which is exactly what p@v needs;
    the output arrives in [HD, S] layout, which is the lhsT layout the
    final wo matmul wants.
"""

import sys

sys.path.insert(0, "/opt/trn_rl_repo")

import math
from dataclasses import dataclass

import numpy as np

import concourse.bass as bass
import concourse.mybir as mybir
import concourse.tile as tile
from concourse import bacc
from concourse.masks import make_identity

f32 = mybir.dt.float32
f32r = mybir.dt.float32r
bf16 = mybir.dt.bfloat16

P = 128


@dataclass(frozen=True)
class Cfg:
    S: int = 2048  # sequence length
    D: int = 4096  # model dim
    HPC: int = 8  # heads per core
    HD: int = 128  # head dim
    AL: int = 10  # adapter len
    mm_dt: object = bf16  # matmul operand dtype
    pipe_depth: int = 1  # phase-2 software pipeline depth

    @property
    def DC(self):  # D chunks of 128 (contraction)
        return self.D // P

    @property
    def DH(self):  # head-slice width
        return self.HPC * self.HD

    @property
    def NQ(self):  # 128-row q blocks
        return self.S // P

    @property
    def NT(self):  # 512-col tiles
        return self.S // 512

    @property
    def XS(self):  # phase-1 x s-tile width
        return 512


def build_nc(cfg: Cfg, phases=(1, 2, 3)):
    nc = bacc.Bacc(None, target_bir_lowering=False, debug=False)
    S, D, HPC, HD, AL = cfg.S, cfg.D, cfg.HPC, cfg.HD, cfg.AL
    DC, DH, NQ, NT, XS = cfg.DC, cfg.DH, cfg.NQ, cfg.NT, cfg.XS
    mdt = cfg.mm_dt
    inv_sqrt = 1.0 / math.sqrt(HD)
    HH = HD // 2

    # ---- I/O ----
    # All big operands arrive host-pre-arranged in partition-major layouts
    # so every DMA is one large contiguous line per partition.
    xT_d = nc.dram_tensor("xT", [P, DC, S], mdt, kind="ExternalInput")
    wq_d = nc.dram_tensor("wq", [P, HPC, DC, HD], mdt, kind="ExternalInput")
    wk_d = nc.dram_tensor("wk", [P, HPC, DC, HD], mdt, kind="ExternalInput")
    wv_d = nc.dram_tensor("wv", [P, HPC, DC, HD], mdt, kind="ExternalInput")
    wo_d = nc.dram_tensor("wo", [P, D // 512, DH // P, 512], mdt, kind="ExternalInput")
    adT_d = nc.dram_tensor("adT", [P, DC, AL], mdt, kind="ExternalInput")
    cosT_d = nc.dram_tensor("cosT", [HH, S], f32, kind="ExternalInput")
    sinT_d = nc.dram_tensor("sinT", [HH, S], f32, kind="ExternalInput")
    # 0/1 upper-triangular (incl diag) [k,q] mask for the boundary block
    tri_d = nc.dram_tensor("tri", [P, P], f32, kind="ExternalInput")
    gate_d = nc.dram_tensor("gateb", [HPC, P, 1], f32, kind="ExternalInput")
    y_d = nc.dram_tensor("y", [S, D], f32, kind="ExternalOutput")

    ExpF = mybir.ActivationFunctionType.Exp
    AX = mybir.AxisListType.X
    Mul = mybir.AluOpType.mult

    with tile.TileContext(nc) as tc:
        with tc.tile_pool(name="persist", bufs=1) as persist:
            # persistent small tiles (cos on partitions 0:64, sin on 64:128)
            # -- loaded via gpsimd so the first xt/wt loads own the HWDGE
            # queues from t=0
            cs_sb = persist.tile([P, S], f32)
            nc.gpsimd.dma_start(cs_sb[0:HH, :], cosT_d[:])
            nc.gpsimd.dma_start(cs_sb[HH:, :], sinT_d[:])
            g_all = persist.tile([P, HPC], f32)
            for h in range(HPC):
                nc.gpsimd.dma_start(g_all[:, h : h + 1], gate_d[h])
            adT_sb = persist.tile([P, DC, AL], mdt)
            nc.gpsimd.dma_start(adT_sb[:], adT_d[:])
            tri_b = persist.tile([P, P], mdt)
            nc.gpsimd.dma_start(tri_b[:], tri_d[:])  # gpsimd DMA casts f32->bf16
            ident_b = persist.tile([P, P], mdt)
            make_identity(nc, ident_b)
            ones_f = persist.tile([P, 1], f32)
            nc.vector.memset(ones_f[:], 1.0)
            ones_c = persist.tile([P, 1], mdt)
            nc.vector.tensor_copy(ones_c[:], ones_f[:])
            ones_r1 = persist.tile([1, P], mdt)
            nc.vector.memset(ones_r1[:], 1.0)

            # persistent per-head state: q/k/v stay in SBUF for the whole
            # kernel; the attention output lives in a pool that spans
            # phases 2-3 only (reusing phase-1's x/w space)
            qT_sb = persist.tile([P, HPC, S], mdt)
            kT_sb = persist.tile([P, HPC, S], mdt)
            vT_sb = persist.tile([P, HPC, S], mdt)
            akT_all = persist.tile([P, HPC, AL], mdt)
            av_all = persist.tile([AL, HPC, P], mdt)

            # ================= Phase 1: projections + rope =================
            NXS = S // XS if 1 in phases else 0
            with (
                tc.tile_pool(name="p1x", bufs=2) as p1x,
                tc.tile_pool(name="p1w", bufs=3) as p1w,
                tc.tile_pool(name="p1t", bufs=1) as p1t,
                tc.tile_pool(name="p1ps", bufs=6, space="PSUM") as p1ps,
                tc.tile_pool(name="p1psa", bufs=1, space="PSUM") as p1psa,
            ):
                for st in range(NXS):
                    soff = st * XS
                    xt = p1x.tile([P, DC, XS], mdt, tag="xt")
                    # two half-loads so chunk-0 matmuls start at half latency
                    HC = DC // 2
                    nc.sync.dma_start(
                        xt[:, 0:HC, :], xT_d[:, 0:HC, soff : soff + XS]
                    )
                    nc.sync.dma_start(
                        xt[:, HC:, :], xT_d[:, HC:, soff : soff + XS]
                    )
                    for h in range(HPC):
                        for pi, (proj, w_dram, dst) in enumerate(
                            (
                                ("q", wq_d, qT_sb),
                                ("k", wk_d, kT_sb),
                                ("v", wv_d, vT_sb),
                            )
                        ):
                            wt = p1w.tile([P, DC, HD], mdt, tag="wt")
                            wq_eng = nc.scalar if (h * 3 + pi) % 2 == 0 else nc.sync
                            wq_eng.dma_start(wt[:], w_dram[:, h])
                            # adapter projections (once, while w resident)
                            if st == 0 and proj == "k":
                                ps_ak = p1psa.tile([P, AL], f32, tag="ps_ak")
                                for c in range(DC):
                                    nc.tensor.matmul(
                                        ps_ak[:],
                                        wt[:, c, :],
                                        adT_sb[:, c, :],
                                        start=(c == 0),
                                        stop=(c == DC - 1),
                                    )
                                nc.vector.tensor_copy(
                                    akT_all[:, h, :], ps_ak[:]
                                )
                            if st == 0 and proj == "v":
                                ps_av = p1psa.tile([AL, P], f32, tag="ps_av")
                                for c in range(DC):
                                    nc.tensor.matmul(
                                        ps_av[:],
                                        adT_sb[:, c, :],
                                        wt[:, c, :],
                                        start=(c == 0),
                                        stop=(c == DC - 1),
                                    )
                                nc.vector.tensor_copy(av_all[:, h, :], ps_av[:])

                            psum = p1ps.tile([P, XS], f32, tag="p1psum")
                            for c in range(DC):
                                nc.tensor.matmul(
                                    psum[:],
                                    wt[:, c, :],
                                    xt[:, c, :],
                                    start=(c == 0),
                                    stop=(c == DC - 1),
                                )
                            if proj == "v":
                                # (gpsimd cannot read PSUM on HW)
                                nc.vector.tensor_copy(
                                    vT_sb[:, h, soff : soff + XS], psum[:]
                                )
                            else:
                                # rope: psum partitions 0:64 = even dims (x0),
                                # 64:128 = odd dims (x1). The four products go
                                # to base-0 tmp tiles (PSUM x SBUF inputs may
                                # differ in base partition; SBUF x SBUF may
                                # not), the two combines are base-aligned and
                                # write straight into the persistent q/k tile.
                                c_ap = cs_sb[0:HH, soff : soff + XS]
                                s_ap = cs_sb[HH:, soff : soff + XS]
                                x0 = psum[0:HH, :]
                                x1 = psum[HH : 2 * HH, :]
                                ta = p1t.tile([HH, XS], f32, tag="ta")
                                tb = p1t.tile([HH, XS], f32, tag="tb")
                                tc2 = p1t.tile([HH, XS], f32, tag="tc")
                                td = p1t.tile([HH, XS], f32, tag="td")
                                nc.vector.tensor_tensor(ta[:], x0, c_ap, op=Mul)
                                nc.vector.tensor_tensor(tb[:], x1, s_ap, op=Mul)
                                nc.vector.tensor_sub(
                                    dst[0:HH, h, soff : soff + XS], ta[:], tb[:]
                                )
                                nc.vector.tensor_tensor(tc2[:], x0, s_ap, op=Mul)
                                nc.vector.tensor_tensor(td[:], x1, c_ap, op=Mul)
                                nc.vector.tensor_add(
                                    dst[HH:, h, soff : soff + XS], tc2[:], td[:]
                                )

            # ================= Phase 2: attention per head =================
            # scoresT layout [k, q]: p = exp(kT_blk.T @ qT_tile * inv_sqrt)
            # lands directly in the layout p@v needs -- no p transposes.
            # Scores are O(5) so exp needs no max subtraction; causal masking
            # multiplies the diagonal-band blocks by a 0/1 mask (on Pool).
            # Full k-blocks are exp'd two at a time (halves ACT per-instr
            # overhead); per-q sums first collapse 4 blocks at a time into
            # bf16 group tiles on the DVE (4x mode), so the PE ones-row
            # matmul streams 3.4x fewer rows. Normalization happens at
            # eviction via a K=1 broadcast matmul of 1/sums.
            # Phases 2+3 are interleaved Q-major: for each 512-wide q tile
            # (round), all 8 heads' attention runs, with the PREVIOUS
            # round's output-projection (y = oT @ wo) chunks slotted in
            # between head blocks -- the PE does phase-3 matmuls while the
            # ACT engine catches up on this round's exps.
            HPC2 = HPC if 2 in phases else 0
            with tc.tile_pool(name="p23o", bufs=1) as p23o:
              # attention output, alive through phases 2+3 (phase-1 space)
              oT_sb = p23o.tile([P, HPC, S], mdt)
              # prefetch phase-3's first wo tile while phase 2 runs
              WC = DH // P
              wo0 = p23o.tile([P, WC, 512], mdt)
              nc.sync.dma_start(wo0[:], wo_d[:, 0])
              with (
                tc.tile_pool(name="p2vn", bufs=2) as p2vn,
                tc.tile_pool(name="p2pt", bufs=cfg.pipe_depth + 1) as p2pt,
                tc.tile_pool(name="p2pg", bufs=cfg.pipe_depth + 1) as p2pg,
                tc.tile_pool(name="p2sm", bufs=3) as p2sm,
                tc.tile_pool(name="p2ad", bufs=2) as p2ad,
                tc.tile_pool(name="p2o", bufs=1) as p2o,
                tc.tile_pool(name="p2ps_s", bufs=2, space="PSUM") as p2ps_s,
                tc.tile_pool(name="p2ps_o", bufs=1, space="PSUM") as p2ps_o,
                tc.tile_pool(name="p2ps_t", bufs=1, space="PSUM") as p2ps_t,
                tc.tile_pool(name="p2ps_b", bufs=2, space="PSUM") as p2ps_b,
              ):

                def emit_pv(ph, pQ, ptb, psg, apT, v_nat):
                    """sums + normalize-broadcast + p@v + adapter + evict.

                    The p@v matmuls are issued between the sums matmul and
                    the broadcast matmul so the DVE reciprocal overlaps PE
                    work instead of stalling it.
                    """
                    nkb = (pQ + 1) * 4
                    ps_su = p2ps_b.tile([1, 512], f32, tag="ps_b")
                    # sums: full-chunk group tiles + the 4 boundary blocks
                    # raw (their DVE pre-sum chain costs more than the PE
                    # rows here, and the PE has idle slack in phase 2)
                    nsu = pQ + 4
                    idx = 0
                    for g in range(pQ):
                        nc.tensor.matmul(
                            ps_su[:],
                            ones_c[:],
                            psg[:, g, :],
                            start=(idx == 0),
                            stop=False,
                        )
                        idx += 1
                    for j in range(4):
                        jb = 4 * pQ + j
                        off = j * P
                        nc.tensor.matmul(
                            ps_su[:, off:],
                            ones_c[:],
                            ptb[:, jb, off:],
                            start=(idx == 0),
                            stop=(idx == nsu - 1),
                        )
                        idx += 1
                    rrow = p2sm.tile([1, 512], mdt, tag="rrow")
                    with nc.allow_low_precision(reason="bf16 softmax recip"):
                        nc.vector.reciprocal(rrow[:], ps_su[:])
                    ps_o = p2ps_o.tile([P, 512], f32, tag="ps_o")
                    for jb in range(nkb):
                        off = 0 if jb < 4 * pQ else (jb - 4 * pQ) * P
                        nc.tensor.matmul(
                            ps_o[:, off:],
                            v_nat[:, jb, :],
                            ptb[:, jb, off:],
                            start=(jb == 0),
                            stop=(jb == nkb - 1),
                        )
                    ps_bc = p2ps_b.tile([P, 512], f32, tag="ps_b")
                    nc.tensor.matmul(
                        ps_bc[:], ones_r1[:], rrow[:], start=True, stop=True
                    )
                    ps_a2 = p2ps_b.tile([P, 512], f32, tag="ps_b")
                    nc.tensor.matmul(
                        ps_a2[:], av_all[:, ph, :], apT[:], start=True, stop=True
                    )
                    bc_sb = p2o.tile([P, 512], f32, tag="bc_sb")
                    nc.any.tensor_copy(bc_sb[:], ps_bc[:])
                    o_ap = oT_sb[:, ph, pQ * 512 : (pQ + 1) * 512]
                    nc.vector.scalar_tensor_tensor(
                        o_ap, ps_o[:], 1.0, bc_sb[:], op0=Mul, op1=Mul
                    )
                    nc.vector.tensor_add(o_ap, o_ap, ps_a2[:])

                pending = []
                for h in range(HPC2):
                    qT = qT_sb[:, h, :]
                    kT = kT_sb[:, h, :]
                    akT = akT_all[:, h, :]

                    # v natural layout [s-block, NQ, d] via one DMA-XBAR
                    # transpose per head (off the PE entirely)
                    v_nat = p2vn.tile([P, NQ, P], mdt, tag="v_nat")
                    nc.sync.dma_start(v_nat[:], vT_sb[:, h, :], transpose=True)

                    # adapter softmax for the whole head in one chain
                    # (natural [q, AL] layout; tiny free sizes, so batching
                    # all 16 q-blocks amortizes the per-op overhead)
                    ps_a16 = p2ps_t.tile([P, NQ, AL], f32, tag="ps_t")
                    for qb in range(NQ):
                        nc.tensor.matmul(
                            ps_a16[:, qb, :],
                            qT[:, qb * P : (qb + 1) * P],
                            akT[:],
                            start=True,
                            stop=True,
                        )
                    asm16 = p2ad.tile([P, NQ, AL], f32, tag="asm")
                    nc.scalar.activation(
                        asm16[:], ps_a16[:], ExpF, bias=0.0, scale=inv_sqrt
                    )
                    asum16 = p2ad.tile([P, NQ], f32, tag="asum")
                    nc.vector.reduce_sum(out=asum16[:], in_=asm16[:], axis=AX)
                    arec16 = p2ad.tile([P, NQ], f32, tag="arec")
                    nc.vector.reciprocal(arec16[:], asum16[:])
                    nc.vector.tensor_tensor(
                        arec16[:],
                        arec16[:],
                        g_all[:, h : h + 1].to_broadcast([P, NQ]),
                        op=Mul,
                    )
                    asm16b = p2ad.tile([P, NQ, AL], mdt, tag="asmb")
                    nc.vector.tensor_tensor(
                        asm16b[:],
                        asm16[:],
                        arec16[:, :, None].to_broadcast([P, NQ, AL]),
                        op=Mul,
                    )

                    for Q in range(NT):
                        nkb = (Q + 1) * 4
                        nfull = 4 * Q
                        qtile = qT[:, Q * 512 : (Q + 1) * 512]
                        ptb = p2pt.tile([P, NQ, 512], mdt, tag="ptb")
                        psg = p2pg.tile([P, NT, 512], mdt, tag="psg")
                        apT = p2sm.tile([AL, 512], mdt, tag="apT")
                        # full k-blocks: scores in pairs, one exp per pair
                        for pj in range(nfull // 2):
                            jb = 2 * pj
                            ps_s2 = p2ps_s.tile([P, 2, 512], f32, tag="ps_s")
                            for half in range(2):
                                nc.tensor.matmul(
                                    ps_s2[:, half, :],
                                    kT[:, (jb + half) * P : (jb + half + 1) * P],
                                    qtile[:],
                                    start=True,
                                    stop=True,
                                )
                            nc.scalar.activation(
                                ptb[:, jb : jb + 2, :],
                                ps_s2[:],
                                ExpF,
                                bias=0.0,
                                scale=inv_sqrt,
                            )
                        # boundary band: 4 blocks exp'd as two pairs at the
                        # wider pair member's range (the narrower member's
                        # left margin exps stale-but-finite psum scores that
                        # nothing ever reads); triangle mask on Pool
                        for bp in range(2):
                            jb = nfull + 2 * bp
                            poff = 2 * bp * P  # pair exp starts here
                            ps_s2 = p2ps_s.tile([P, 2, 512], f32, tag="ps_s")
                            for half in range(2):
                                off = (2 * bp + half) * P
                                nc.tensor.matmul(
                                    ps_s2[:, half, off:],
                                    kT[:, (jb + half) * P : (jb + half + 1) * P],
                                    qtile[:, off:],
                                    start=True,
                                    stop=True,
                                )
                            nc.scalar.activation(
                                ptb[:, jb : jb + 2, poff:],
                                ps_s2[:, :, poff:],
                                ExpF,
                                bias=0.0,
                                scale=inv_sqrt,
                            )
                            for half in range(2):
                                off = (2 * bp + half) * P
                                nc.gpsimd.tensor_mul(
                                    ptb[:, jb + half, off : off + P],
                                    ptb[:, jb + half, off : off + P],
                                    tri_b[:],
                                )
                        # grouped block sums (bf16, DVE 4x): 4 full blocks
                        # per group; boundary blocks are summed raw by the
                        # PE ones-matmul in emit_pv
                        for g in range(Q):
                            b = 4 * g
                            nc.vector.tensor_add(
                                psg[:, g, :], ptb[:, b, :], ptb[:, b + 1, :]
                            )
                            nc.vector.tensor_add(
                                psg[:, g, :], psg[:, g, :], ptb[:, b + 2, :]
                            )
                            nc.vector.tensor_add(
                                psg[:, g, :], psg[:, g, :], ptb[:, b + 3, :]
                            )
                        # pipeline: heavy tail of an OLDER q-tile before the
                        # adapter chain, so PE stays fed while the newer
                        # tile's exps run on ACT
                        if len(pending) >= cfg.pipe_depth:
                            emit_pv(*pending.pop(0))

                        # adapter probs for this q tile: transpose the
                        # head-level normalized probs into [AL, q]
                        for qb in range(4):
                            ps_apt = p2ps_t.tile([P, P], mdt, tag="ps_t")
                            nc.tensor.transpose(
                                ps_apt[:AL, :],
                                asm16b[:, Q * 4 + qb, :],
                                ident_b[:],
                            )
                            nc.any.tensor_copy(
                                apT[:, qb * P : (qb + 1) * P], ps_apt[:AL, :]
                            )
                        pending.append((h, Q, ptb, psg, apT, v_nat))
                for entry in pending:
                    emit_pv(*entry)
                pending = []

              # ================= Phase 3: out @ wo =================
              with (
                tc.tile_pool(name="p3w", bufs=2) as p3w,
                tc.tile_pool(name="p3y", bufs=3) as p3y,
                tc.tile_pool(name="p3ps", bufs=4, space="PSUM") as p3ps,
              ):
                for et in range(D // 512 if 3 in phases else 0):
                    if et == 0:
                        wo_t = wo0
                    else:
                        wo_t = p3w.tile([P, WC, 512], mdt, tag="wo_t")
                        nc.scalar.dma_start(wo_t[:], wo_d[:, et])
                    for st in range(NQ):
                        ps_y = p3ps.tile([P, 512], f32, tag="ps_y")
                        for h in range(HPC):
                            nc.tensor.matmul(
                                ps_y[:],
                                oT_sb[:, h, st * P : (st + 1) * P],
                                wo_t[:, h, :],
                                start=(h == 0),
                                stop=(h == HPC - 1),
                            )
                        y_sb = p3y.tile([P, 512], f32, tag="y_sb")
                        nc.scalar.copy(y_sb[:], ps_y[:])
                        nc.sync.dma_start(
                            y_d[st * P : (st + 1) * P, et * 512 : (et + 1) * 512],
                            y_sb[:],
                        )

    nc.compile()
    return nc


# ====================== host side: sharding + runner ======================

B, S, D, H = 2, 2048, 4096, 32
HD = D // H
AL = 10
N_CORES = 8
TP = 4  # head groups
HPC = H // TP  # 8 heads per core

_RUNNER = None


def _make_runner(nc, n_cores=N_CORES):
    import jax
    from jax.sharding import Mesh, PartitionSpec
    from jax.experimental.shard_map import shard_map

    from concourse import bass2jax
    from concourse.bass2jax import _bass_exec_p, install_neuronx_cc_hook

    install_neuronx_cc_hook()
    partition_name = nc.partition_id_tensor.name if nc.partition_id_tensor else None

    in_names, out_names, out_avals = [], [], []
    for alloc in nc.m.functions[0].allocations:
        if not isinstance(alloc, mybir.MemoryLocationSet):
            continue
        name = alloc.memorylocations[0].name
        if alloc.kind == "ExternalInput":
            if name != partition_name:
                in_names.append(name)
        elif alloc.kind == "ExternalOutput":
            out_names.append(name)
            out_avals.append(
                jax.core.ShapedArray(
                    tuple(alloc.tensor_shape), mybir.dt.np(alloc.dtype)
                )
            )
    n_params = len(in_names)
    n_outs = len(out_avals)
    all_in_names = list(in_names) + list(out_names)
    if partition_name is not None:
        all_in_names.append(partition_name)

    def _body(*args):
        operands = list(args)
        if partition_name is not None:
            operands.append(bass2jax.partition_id_tensor())
        outs = _bass_exec_p.bind(
            *operands,
            out_avals=tuple(out_avals),
            in_names=tuple(all_in_names),
            out_names=tuple(out_names),
            lowering_input_output_aliases=(),
            sim_require_finite=True,
            sim_require_nnan=True,
            nc=nc,
        )
        return tuple(outs)

    devices = jax.devices()[:n_cores]
    mesh = Mesh(np.asarray(devices), ("core",))
    fn = jax.jit(
        shard_map(
            _body,
            mesh=mesh,
            in_specs=(PartitionSpec("core"),) * (n_params + n_outs),
            out_specs=(PartitionSpec("core"),) * n_outs,
            check_rep=False,
        ),
        keep_unused=True,
    )

    class Runner:
        in_names_ = in_names
        out_names_ = out_names

        def prep(self, in_maps):
            import jax as _jax

            concat_in = [
                np.concatenate(
                    [np.ascontiguousarray(in_maps[c][n]) for c in range(n_cores)],
                    axis=0,
                )
                for n in in_names
            ]
            concat_zero = [
                np.zeros((n_cores * a.shape[0], *a.shape[1:]), a.dtype)
                for a in out_avals
            ]
            shardings = [
                _jax.sharding.NamedSharding(mesh, PartitionSpec("core"))
            ] * (n_params + n_outs)
            return _jax.device_put(concat_in + concat_zero, shardings)

        def run(self, args):
            import jax as _jax

            outs = fn(*args)
            _jax.block_until_ready(outs)
            return [
                {
                    n: np.asarray(outs[i]).reshape(n_cores, *out_avals[i].shape)[c]
                    for i, n in enumerate(out_names)
                }
                for c in range(n_cores)
            ]

        def time_pipelined(self, args, reps=10, warmup=1):
            import time as _time

            import jax as _jax

            for _ in range(warmup):
                _jax.block_until_ready(fn(*args))
            t0 = _time.perf_counter()
            outs = None
            for _ in range(reps):
                outs = fn(*args)
            _jax.block_until_ready(outs)
            return (_time.perf_counter() - t0) / reps

    return Runner()


def _shard_inputs(x, cos, sin, mask, wq, wk, wv, wo, gate, adapter):
    """Build the 8 per-core input maps."""
    import ml_dtypes

    bf = ml_dtypes.bfloat16
    # rope permutation of head-dim columns: even dims first, odd second
    perm = np.concatenate(
        [np.arange(0, HD, 2), np.arange(1, HD, 2)]
    )  # within one head
    col_perm = np.concatenate(
        [h * HD + perm for h in range(H)]
    )  # all heads, head-major
    wq_p = np.asarray(wq, dtype=np.float32)[:, col_perm].astype(bf)
    wk_p = np.asarray(wk, dtype=np.float32)[:, col_perm].astype(bf)
    wv_b = np.asarray(wv, dtype=np.float32).astype(bf)
    wo_b = np.asarray(wo, dtype=np.float32).astype(bf)

    DC = D // P
    HPC_ = HPC
    WC = HPC_ * HD // P

    def _prearrange_w(w_slice):
        # [D, DH] -> [P, HPC, DC, HD]: contiguous per-partition head tiles
        return np.ascontiguousarray(
            w_slice.reshape(DC, P, HPC_, HD).transpose(1, 2, 0, 3)
        )

    def _prearrange_wo(wo_slice):
        # [DH, D] -> [P, D//512, WC, 512]
        return np.ascontiguousarray(
            wo_slice.reshape(WC, P, D // 512, 512).transpose(1, 2, 0, 3)
        )

    cosT = np.ascontiguousarray(cos.T, dtype=np.float32)  # [64, S]
    sinT = np.ascontiguousarray(sin.T, dtype=np.float32)
    adT = (
        np.ascontiguousarray(adapter[0].T, dtype=np.float32)
        .astype(bf)
        .reshape(DC, P, AL)
        .transpose(1, 0, 2)
    )  # [P, DC, AL]
    adT = np.ascontiguousarray(adT)

    # 0/1 [k, q] allowed-mask of an aligned 128x128 diagonal block,
    # derived from the mask input (k <= q allowed)
    m = np.asarray(mask, dtype=np.float32)[0, 0]  # [S, S]
    tri = np.ascontiguousarray((m[:P, :P].T == 0)).astype(np.float32)

    gate_v = np.asarray(gate, dtype=np.float32).reshape(H)  # per head

    xT = [
        np.ascontiguousarray(
            np.asarray(x[b], dtype=np.float32)
            .T.astype(bf)
            .reshape(D // P, P, S)
            .transpose(1, 0, 2)
        )
        for b in range(B)
    ]  # [P, DC, S]

    in_maps = []
    for c in range(N_CORES):
        b = c // TP
        g = c % TP
        hs = g * HPC * HD  # column slice start
        gateb = np.ascontiguousarray(
            np.repeat(gate_v[g * HPC : (g + 1) * HPC, None], P, axis=1)[..., None]
        ).astype(np.float32)  # [HPC, P, 1]
        in_maps.append(
            {
                "xT": xT[b],
                "wq": _prearrange_w(wq_p[:, hs : hs + HPC * HD]),
                "wk": _prearrange_w(wk_p[:, hs : hs + HPC * HD]),
                "wv": _prearrange_w(wv_b[:, hs : hs + HPC * HD]),
                "wo": _prearrange_wo(wo_b[hs : hs + HPC * HD, :]),
                "adT": adT,
                "cosT": cosT,
                "sinT": sinT,
                "tri": tri,
                "gateb": gateb,
            }
        )
    return in_maps


def get_runner():
    global _RUNNER
    if _RUNNER is None:
        nc = build_nc(Cfg())
        _RUNNER = _make_runner(nc)
    return _RUNNER


def kernel(**inputs) -> np.ndarray:
    x = np.asarray(inputs["x"])
    in_maps = _shard_inputs(
        x,
        inputs["cos"],
        inputs["sin"],
        inputs["mask"],
        inputs["wq"],
        inputs["wk"],
        inputs["wv"],
        inputs["wo"],
        inputs["gate"],
        inputs["adapter"],
    )
    runner = get_runner()
    args = runner.prep(in_maps)
    outs = runner.run(args)
    y = np.zeros((B, S, D), dtype=np.float32)
    for c in range(N_CORES):
        y[c // TP] += outs[c]["y"]
    return y



# revision 11
# speedup vs baseline: 1.7181x; 1.7181x over previous
"""Trainium2 Bass kernel for nn_Attention_66563403153646.

Dense transformer attention block with rotary embeddings + gated adapter
(prefix) attention, fp32 reference:

    y = softmax(rope(x@wq) @ rope(x@wk).T * k + mask) @ (x@wv)
      + gate * softmax(rope(x@wq) @ (adapter@wk).T * k) @ (adapter@wv)
    out = y @ wo

Sharding: 4-way tensor-parallel over heads x 2-way data-parallel over batch
(8 NeuronCores). Each core computes a [S, D] partial of its batch's output
(its 8 heads' contribution through wo); the host sums the 4 TP partials.

All matmul operands are bf16 (PE 1 cycle/row); accumulation is fp32 in
PSUM. q/k/v and the attention output stay resident in SBUF for the whole
kernel -- no DRAM scratch round trips. Layouts:
  - x is fed transposed ([D, S]) so projections contract D on partitions;
    it streams through SBUF in 512-column tiles while per-head weight
    tiles are (re)loaded per x-tile.
  - q/k live per-head in [HD, S] layout with rope-pair-permuted head dims
    (host permutes wq/wk columns: even rope dims first, odd second) so
    RoPE is two partition-halves of elementwise ops writing straight into
    the persistent SBUF tiles.
  - scores are computed per 512-col q tile over 128-row k blocks
    (causal: only blocks <= diagonal; the diagonal block gets a 0/1
    triangle mask).
  - p blocks land in [k, q] layout, which is exactly what p@v needs;
    the output arrives in [HD, S] layout, which is the lhsT layout the
    final wo matmul wants.
"""

import sys

sys.path.insert(0, "/opt/trn_rl_repo")

import math
from dataclasses import dataclass

import numpy as np

import concourse.bass as bass
import concourse.mybir as mybir
import concourse.tile as tile
from concourse import bacc
from concourse.masks import make_identity

f32 = mybir.dt.float32
f32r = mybir.dt.float32r
bf16 = mybir.dt.bfloat16

P = 128


@dataclass(frozen=True)
class Cfg:
    S: int = 2048  # sequence length
    D: int = 4096  # model dim
    HPC: int = 8  # heads per core
    HD: int = 128  # head dim
    AL: int = 10  # adapter len
    mm_dt: object = bf16  # matmul operand dtype
    pipe_depth: int = 1  # phase-2 software pipeline depth

    @property
    def DC(self):  # D chunks of 128 (contraction)
        return self.D // P

    @property
    def DH(self):  # head-slice width
        return self.HPC * self.HD

    @property
    def NQ(self):  # 128-row q blocks
        return self.S // P

    @property
    def NT(self):  # 512-col tiles
        return self.S // 512

    @property
    def XS(self):  # phase-1 x s-tile width
        return 512


def build_nc(cfg: Cfg, phases=(1, 2, 3)):
    nc = bacc.Bacc(None, target_bir_lowering=False, debug=False, num_devices=8)
    S, D, HPC, HD, AL = cfg.S, cfg.D, cfg.HPC, cfg.HD, cfg.AL
    DC, DH, NQ, NT, XS = cfg.DC, cfg.DH, cfg.NQ, cfg.NT, cfg.XS
    mdt = cfg.mm_dt
    inv_sqrt = 1.0 / math.sqrt(HD)
    HH = HD // 2
    # phase-3 tensor-parallel pair structure: cores (2i, 2i+1) each compute
    # HALF the output columns for BOTH cores' heads (o exchanged via a
    # per-head pairwise AllGather). WC3 = contraction chunks (16 = 2x8 heads),
    # ET3 = 512-col output tiles per core (half of D).
    WC3 = 2 * HPC
    ET3 = D // 512 // 2
    PAIR_GROUPS = [[0, 1], [2, 3], [4, 5], [6, 7]]

    # ---- I/O ----
    # All big operands arrive host-pre-arranged in partition-major layouts
    # so every DMA is one large contiguous line per partition.
    xT_d = nc.dram_tensor("xT", [P, DC, S], mdt, kind="ExternalInput")
    wq_d = nc.dram_tensor("wq", [P, HPC, DC, HD], mdt, kind="ExternalInput")
    wk_d = nc.dram_tensor("wk", [P, HPC, DC, HD], mdt, kind="ExternalInput")
    wv_d = nc.dram_tensor("wv", [P, HPC, DC, HD], mdt, kind="ExternalInput")
    wo_d = nc.dram_tensor("wo", [P, ET3, WC3, 512], mdt, kind="ExternalInput")
    # host-computed adapter projections (tiny): akT in rope-permuted basis,
    # av with the per-head gate folded in
    akT_d = nc.dram_tensor("akT", [P, HPC, AL], mdt, kind="ExternalInput")
    av_d = nc.dram_tensor("av", [AL, HPC, P], mdt, kind="ExternalInput")
    cosT_d = nc.dram_tensor("cosT", [HH, S], f32, kind="ExternalInput")
    sinT_d = nc.dram_tensor("sinT", [HH, S], f32, kind="ExternalInput")
    # 0/1 upper-triangular (incl diag) [k,q] mask for the boundary block
    tri_d = nc.dram_tensor("tri", [P, P], f32, kind="ExternalInput")
    y_d = nc.dram_tensor("y", [S, D // 2], f32, kind="ExternalOutput")

    ExpF = mybir.ActivationFunctionType.Exp
    AX = mybir.AxisListType.X
    Mul = mybir.AluOpType.mult

    with tile.TileContext(nc) as tc:
        with (
            tc.tile_pool(name="persist", bufs=1) as persist,
            tc.tile_pool(name="ccdram", bufs=1, space="DRAM") as ccdram,
        ):
            # DRAM bounce buffers for the per-head o AllGather (collectives
            # must be HBM-to-HBM); per-head slices are contiguous
            cc_in = ccdram.tile([HPC, P, S], mdt)
            cc_out = ccdram.tile([HPC, 2, P, S], mdt)
            # persistent small tiles (cos on partitions 0:64, sin on 64:128)
            # -- loaded via gpsimd so the first xt/wt loads own the HWDGE
            # queues from t=0
            cs_sb = persist.tile([P, S], f32)
            nc.gpsimd.dma_start(cs_sb[0:HH, :], cosT_d[:])
            nc.gpsimd.dma_start(cs_sb[HH:, :], sinT_d[:])
            tri_b = persist.tile([P, P], mdt)
            nc.gpsimd.dma_start(tri_b[:], tri_d[:])  # gpsimd DMA casts f32->bf16
            ident_b = persist.tile([P, P], mdt)
            make_identity(nc, ident_b)
            ones_f = persist.tile([P, 1], f32)
            nc.vector.memset(ones_f[:], 1.0)
            ones_c = persist.tile([P, 1], mdt)
            nc.vector.tensor_copy(ones_c[:], ones_f[:])
            ones_r1 = persist.tile([1, P], mdt)
            nc.vector.memset(ones_r1[:], 1.0)

            # persistent per-head state: q/k/v stay in SBUF for the whole
            # kernel
            qT_sb = persist.tile([P, HPC, S], mdt)
            kT_sb = persist.tile([P, HPC, S], mdt)
            vT_sb = persist.tile([P, HPC, S], mdt)
            akT_all = persist.tile([P, HPC, AL], mdt)
            av_all = persist.tile([AL, HPC, P], mdt)
            nc.gpsimd.dma_start(akT_all[:], akT_d[:])
            nc.gpsimd.dma_start(av_all[:], av_d[:])

            # ================= Phase 1: projections + rope =================
            NXS = S // XS if 1 in phases else 0
            with (
                tc.tile_pool(name="p1x", bufs=2) as p1x,
                tc.tile_pool(name="p1w", bufs=3) as p1w,
                tc.tile_pool(name="p1t", bufs=1) as p1t,
                tc.tile_pool(name="p1ps", bufs=6, space="PSUM") as p1ps,
            ):
                for st in range(NXS):
                    soff = st * XS
                    xt = p1x.tile([P, DC, XS], mdt, tag="xt")
                    # two half-loads so chunk-0 matmuls start at half latency
                    HC = DC // 2
                    nc.sync.dma_start(
                        xt[:, 0:HC, :], xT_d[:, 0:HC, soff : soff + XS]
                    )
                    nc.sync.dma_start(
                        xt[:, HC:, :], xT_d[:, HC:, soff : soff + XS]
                    )
                    for h in range(HPC):
                        for pi, (proj, w_dram, dst) in enumerate(
                            (
                                ("q", wq_d, qT_sb),
                                ("k", wk_d, kT_sb),
                                ("v", wv_d, vT_sb),
                            )
                        ):
                            wt = p1w.tile([P, DC, HD], mdt, tag="wt")
                            wq_eng = nc.scalar if (h * 3 + pi) % 2 == 0 else nc.sync
                            wq_eng.dma_start(wt[:], w_dram[:, h])

                            psum = p1ps.tile([P, XS], f32, tag="p1psum")
                            for c in range(DC):
                                nc.tensor.matmul(
                                    psum[:],
                                    wt[:, c, :],
                                    xt[:, c, :],
                                    start=(c == 0),
                                    stop=(c == DC - 1),
                                )
                            if proj == "v":
                                # (gpsimd cannot read PSUM on HW)
                                nc.vector.tensor_copy(
                                    vT_sb[:, h, soff : soff + XS], psum[:]
                                )
                            else:
                                # rope: psum partitions 0:64 = even dims (x0),
                                # 64:128 = odd dims (x1). The four products go
                                # to base-0 tmp tiles (PSUM x SBUF inputs may
                                # differ in base partition; SBUF x SBUF may
                                # not), the two combines are base-aligned and
                                # write straight into the persistent q/k tile.
                                c_ap = cs_sb[0:HH, soff : soff + XS]
                                s_ap = cs_sb[HH:, soff : soff + XS]
                                x0 = psum[0:HH, :]
                                x1 = psum[HH : 2 * HH, :]
                                ta = p1t.tile([HH, XS], f32, tag="ta")
                                tb = p1t.tile([HH, XS], f32, tag="tb")
                                tc2 = p1t.tile([HH, XS], f32, tag="tc")
                                td = p1t.tile([HH, XS], f32, tag="td")
                                nc.vector.tensor_tensor(ta[:], x0, c_ap, op=Mul)
                                nc.vector.tensor_tensor(tb[:], x1, s_ap, op=Mul)
                                nc.vector.tensor_sub(
                                    dst[0:HH, h, soff : soff + XS], ta[:], tb[:]
                                )
                                nc.vector.tensor_tensor(tc2[:], x0, s_ap, op=Mul)
                                nc.vector.tensor_tensor(td[:], x1, c_ap, op=Mul)
                                nc.vector.tensor_add(
                                    dst[HH:, h, soff : soff + XS], tc2[:], td[:]
                                )

            # ================= Phase 2: attention per head =================
            # scoresT layout [k, q]: p = exp(kT_blk.T @ qT_tile * inv_sqrt)
            # lands directly in the layout p@v needs -- no p transposes.
            # Scores are O(5) so exp needs no max subtraction; causal masking
            # multiplies the diagonal-band blocks by a 0/1 mask (on Pool).
            # Full k-blocks are exp'd two at a time (halves ACT per-instr
            # overhead); per-q sums first collapse 4 blocks at a time into
            # bf16 group tiles on the DVE (4x mode), so the PE ones-row
            # matmul streams 3.4x fewer rows. Normalization happens at
            # eviction via a K=1 broadcast matmul of 1/sums.
            # Phases 2+3 are interleaved Q-major: for each 512-wide q tile
            # (round), all 8 heads' attention runs, with the PREVIOUS
            # round's output-projection (y = oT @ wo) chunks slotted in
            # between head blocks -- the PE does phase-3 matmuls while the
            # ACT engine catches up on this round's exps.
            HPC2 = HPC if 2 in phases else 0
            with tc.tile_pool(name="p23o", bufs=1) as p23o:
              # prefetch phase-3's first wo tile while phase 2 runs
              wo0 = p23o.tile([P, WC3, 512], mdt)
              nc.sync.dma_start(wo0[:], wo_d[:, 0])
              with (
                tc.tile_pool(name="p2vn", bufs=2) as p2vn,
                tc.tile_pool(name="p2pt", bufs=cfg.pipe_depth + 1) as p2pt,
                tc.tile_pool(name="p2pg", bufs=cfg.pipe_depth + 1) as p2pg,
                tc.tile_pool(name="p2sm", bufs=3) as p2sm,
                tc.tile_pool(name="p2ad", bufs=2) as p2ad,
                tc.tile_pool(name="p2o", bufs=2) as p2o,
                tc.tile_pool(name="p2ps_s", bufs=2, space="PSUM") as p2ps_s,
                tc.tile_pool(name="p2ps_o", bufs=1, space="PSUM") as p2ps_o,
                tc.tile_pool(name="p2ps_t", bufs=1, space="PSUM") as p2ps_t,
                tc.tile_pool(name="p2ps_b", bufs=2, space="PSUM") as p2ps_b,
              ):

                def emit_pv(ph, pQ, ptb, psg, apT, v_nat):
                    """sums + normalize-broadcast + p@v + adapter + evict.

                    The p@v matmuls are issued between the sums matmul and
                    the broadcast matmul so the DVE reciprocal overlaps PE
                    work instead of stalling it.
                    """
                    nkb = (pQ + 1) * 4
                    ps_su = p2ps_b.tile([1, 512], f32, tag="ps_b")
                    # sums: full-chunk group tiles + the 4 boundary blocks
                    # raw (their DVE pre-sum chain costs more than the PE
                    # rows here, and the PE has idle slack in phase 2)
                    nsu = pQ + 4
                    idx = 0
                    for g in range(pQ):
                        nc.tensor.matmul(
                            ps_su[:],
                            ones_c[:],
                            psg[:, g, :],
                            start=(idx == 0),
                            stop=False,
                        )
                        idx += 1
                    for j in range(4):
                        jb = 4 * pQ + j
                        off = j * P
                        nc.tensor.matmul(
                            ps_su[:, off:],
                            ones_c[:],
                            ptb[:, jb, off:],
                            start=(idx == 0),
                            stop=(idx == nsu - 1),
                        )
                        idx += 1
                    rrow = p2sm.tile([1, 512], mdt, tag="rrow")
                    with nc.allow_low_precision(reason="bf16 softmax recip"):
                        nc.vector.reciprocal(rrow[:], ps_su[:])
                    ps_o = p2ps_o.tile([P, 512], f32, tag="ps_o")
                    for jb in range(nkb):
                        off = 0 if jb < 4 * pQ else (jb - 4 * pQ) * P
                        nc.tensor.matmul(
                            ps_o[:, off:],
                            v_nat[:, jb, :],
                            ptb[:, jb, off:],
                            start=(jb == 0),
                            stop=(jb == nkb - 1),
                        )
                    ps_bc = p2ps_b.tile([P, 512], f32, tag="ps_b")
                    nc.tensor.matmul(
                        ps_bc[:], ones_r1[:], rrow[:], start=True, stop=True
                    )
                    ps_a2 = p2ps_b.tile([P, 512], f32, tag="ps_b")
                    nc.tensor.matmul(
                        ps_a2[:], av_all[:, ph, :], apT[:], start=True, stop=True
                    )
                    bc_sb = p2o.tile([P, 512], f32, tag="bc_sb")
                    nc.any.tensor_copy(bc_sb[:], ps_bc[:])
                    o_ev = p2o.tile([P, 512], mdt, tag="o_ev")
                    nc.vector.scalar_tensor_tensor(
                        o_ev[:], ps_o[:], 1.0, bc_sb[:], op0=Mul, op1=Mul
                    )
                    nc.vector.tensor_add(o_ev[:], o_ev[:], ps_a2[:])
                    # stage o into the collective bounce buffer; after the
                    # head's last q-tile, exchange full o with the pair peer
                    nc.gpsimd.dma_start(
                        cc_in[ph, :, pQ * 512 : (pQ + 1) * 512], o_ev[:]
                    )
                    if pQ == NT - 1:
                        nc.gpsimd.collective_compute(
                            "AllGather",
                            mybir.AluOpType.bypass,
                            replica_groups=PAIR_GROUPS,
                            ins=[cc_in[ph].opt()],
                            outs=[cc_out[ph].opt()],
                        )

                pending = []
                for h in range(HPC2):
                    qT = qT_sb[:, h, :]
                    kT = kT_sb[:, h, :]
                    akT = akT_all[:, h, :]

                    # v natural layout [s-block, NQ, d] via one DMA-XBAR
                    # transpose per head (off the PE entirely)
                    v_nat = p2vn.tile([P, NQ, P], mdt, tag="v_nat")
                    nc.sync.dma_start(v_nat[:], vT_sb[:, h, :], transpose=True)

                    # adapter softmax for the whole head in one chain
                    # (natural [q, AL] layout; tiny free sizes, so batching
                    # all 16 q-blocks amortizes the per-op overhead)
                    ps_a16 = p2ps_t.tile([P, NQ, AL], f32, tag="ps_t")
                    for qb in range(NQ):
                        nc.tensor.matmul(
                            ps_a16[:, qb, :],
                            qT[:, qb * P : (qb + 1) * P],
                            akT[:],
                            start=True,
                            stop=True,
                        )
                    asm16 = p2ad.tile([P, NQ, AL], f32, tag="asm")
                    nc.scalar.activation(
                        asm16[:], ps_a16[:], ExpF, bias=0.0, scale=inv_sqrt
                    )
                    asum16 = p2ad.tile([P, NQ], f32, tag="asum")
                    nc.vector.reduce_sum(out=asum16[:], in_=asm16[:], axis=AX)
                    arec16 = p2ad.tile([P, NQ], f32, tag="arec")
                    nc.vector.reciprocal(arec16[:], asum16[:])
                    asm16b = p2ad.tile([P, NQ, AL], mdt, tag="asmb")
                    nc.vector.tensor_tensor(
                        asm16b[:],
                        asm16[:],
                        arec16[:, :, None].to_broadcast([P, NQ, AL]),
                        op=Mul,
                    )

                    for Q in range(NT):
                        nkb = (Q + 1) * 4
                        nfull = 4 * Q
                        qtile = qT[:, Q * 512 : (Q + 1) * 512]
                        ptb = p2pt.tile([P, NQ, 512], mdt, tag="ptb")
                        psg = p2pg.tile([P, NT, 512], mdt, tag="psg")
                        apT = p2sm.tile([AL, 512], mdt, tag="apT")
                        # full k-blocks: scores in pairs, one exp per pair
                        for pj in range(nfull // 2):
                            jb = 2 * pj
                            ps_s2 = p2ps_s.tile([P, 2, 512], f32, tag="ps_s")
                            for half in range(2):
                                nc.tensor.matmul(
                                    ps_s2[:, half, :],
                                    kT[:, (jb + half) * P : (jb + half + 1) * P],
                                    qtile[:],
                                    start=True,
                                    stop=True,
                                )
                            nc.scalar.activation(
                                ptb[:, jb : jb + 2, :],
                                ps_s2[:],
                                ExpF,
                                bias=0.0,
                                scale=inv_sqrt,
                            )
                        # boundary band: 4 blocks exp'd as two pairs at the
                        # wider pair member's range (the narrower member's
                        # left margin exps stale-but-finite psum scores that
                        # nothing ever reads); triangle mask on Pool
                        for bp in range(2):
                            jb = nfull + 2 * bp
                            poff = 2 * bp * P  # pair exp starts here
                            ps_s2 = p2ps_s.tile([P, 2, 512], f32, tag="ps_s")
                            for half in range(2):
                                off = (2 * bp + half) * P
                                nc.tensor.matmul(
                                    ps_s2[:, half, off:],
                                    kT[:, (jb + half) * P : (jb + half + 1) * P],
                                    qtile[:, off:],
                                    start=True,
                                    stop=True,
                                )
                            nc.scalar.activation(
                                ptb[:, jb : jb + 2, poff:],
                                ps_s2[:, :, poff:],
                                ExpF,
                                bias=0.0,
                                scale=inv_sqrt,
                            )
                            for half in range(2):
                                off = (2 * bp + half) * P
                                nc.gpsimd.tensor_mul(
                                    ptb[:, jb + half, off : off + P],
                                    ptb[:, jb + half, off : off + P],
                                    tri_b[:],
                                )
                        # grouped block sums (bf16, DVE 4x): 4 full blocks
                        # per group; boundary blocks are summed raw by the
                        # PE ones-matmul in emit_pv
                        for g in range(Q):
                            b = 4 * g
                            nc.vector.tensor_add(
                                psg[:, g, :], ptb[:, b, :], ptb[:, b + 1, :]
                            )
                            nc.vector.tensor_add(
                                psg[:, g, :], psg[:, g, :], ptb[:, b + 2, :]
                            )
                            nc.vector.tensor_add(
                                psg[:, g, :], psg[:, g, :], ptb[:, b + 3, :]
                            )
                        # pipeline: heavy tail of an OLDER q-tile before the
                        # adapter chain, so PE stays fed while the newer
                        # tile's exps run on ACT
                        if len(pending) >= cfg.pipe_depth:
                            emit_pv(*pending.pop(0))

                        # adapter probs for this q tile: transpose the
                        # head-level normalized probs into [AL, q]
                        for qb in range(4):
                            ps_apt = p2ps_t.tile([P, P], mdt, tag="ps_t")
                            nc.tensor.transpose(
                                ps_apt[:AL, :],
                                asm16b[:, Q * 4 + qb, :],
                                ident_b[:],
                            )
                            nc.any.tensor_copy(
                                apT[:, qb * P : (qb + 1) * P], ps_apt[:AL, :]
                            )
                        pending.append((h, Q, ptb, psg, apT, v_nat))
                for entry in pending:
                    emit_pv(*entry)
                pending = []

              # ================= Phase 3: out @ wo =================
              # Each core computes y[:, its D-half] contracting ALL 16 pair
              # heads (8 local + 8 from the peer via the AllGather). The
              # member asymmetry lives entirely in the host-side wo layout
              # (wc slot = h*2 + j, j = pair-member slot), so the program is
              # SPMD-uniform. o is streamed back from the collective bounce
              # per 128-row s-block.
              with (
                tc.tile_pool(name="p3w", bufs=1) as p3w,
                tc.tile_pool(name="p3o", bufs=3) as p3o,
                tc.tile_pool(name="p3y", bufs=3) as p3y,
                tc.tile_pool(name="p3ps", bufs=4, space="PSUM") as p3ps,
              ):
                # all 4 wo tiles resident (st-outer loop reuses them all)
                wo_ts = [wo0]
                for et in range(1, ET3 if 3 in phases else 0):
                    wo_t = p3w.tile(
                        [P, WC3, 512], mdt, name=f"wo_t{et}", tag=f"wo_t{et}"
                    )
                    nc.scalar.dma_start(wo_t[:], wo_d[:, et])
                    wo_ts.append(wo_t)
                for st in range(NQ if 3 in phases else 0):
                    o_st = p3o.tile([P, HPC, 2, P], mdt, tag="o_st")
                    for h in range(HPC):
                        for j in range(2):
                            eng = nc.sync if (h * 2 + j) % 2 == 0 else nc.scalar
                            eng.dma_start(
                                o_st[:, h, j, :],
                                cc_out[h, j, :, st * P : (st + 1) * P],
                            )
                    for et in range(ET3):
                        ps_y = p3ps.tile([P, 512], f32, tag="ps_y")
                        for w in range(WC3):
                            nc.tensor.matmul(
                                ps_y[:],
                                o_st[:, w // 2, w % 2, :],
                                wo_ts[et][:, w, :],
                                start=(w == 0),
                                stop=(w == WC3 - 1),
                            )
                        y_sb = p3y.tile([P, 512], f32, tag="y_sb")
                        nc.scalar.copy(y_sb[:], ps_y[:])
                        nc.sync.dma_start(
                            y_d[st * P : (st + 1) * P, et * 512 : (et + 1) * 512],
                            y_sb[:],
                        )

    nc.compile()
    return nc


# ====================== host side: sharding + runner ======================

B, S, D, H = 2, 2048, 4096, 32
HD = D // H
AL = 10
N_CORES = 8
TP = 4  # head groups
HPC = H // TP  # 8 heads per core

_RUNNER = None


def _make_runner(nc, n_cores=N_CORES):
    import jax
    from jax.sharding import Mesh, PartitionSpec
    from jax.experimental.shard_map import shard_map

    from concourse import bass2jax
    from concourse.bass2jax import _bass_exec_p, install_neuronx_cc_hook

    install_neuronx_cc_hook()
    partition_name = nc.partition_id_tensor.name if nc.partition_id_tensor else None

    in_names, out_names, out_avals = [], [], []
    for alloc in nc.m.functions[0].allocations:
        if not isinstance(alloc, mybir.MemoryLocationSet):
            continue
        name = alloc.memorylocations[0].name
        if alloc.kind == "ExternalInput":
            if name != partition_name:
                in_names.append(name)
        elif alloc.kind == "ExternalOutput":
            out_names.append(name)
            out_avals.append(
                jax.core.ShapedArray(
                    tuple(alloc.tensor_shape), mybir.dt.np(alloc.dtype)
                )
            )
    n_params = len(in_names)
    n_outs = len(out_avals)
    all_in_names = list(in_names) + list(out_names)
    if partition_name is not None:
        all_in_names.append(partition_name)

    def _body(*args):
        operands = list(args)
        if partition_name is not None:
            operands.append(bass2jax.partition_id_tensor())
        outs = _bass_exec_p.bind(
            *operands,
            out_avals=tuple(out_avals),
            in_names=tuple(all_in_names),
            out_names=tuple(out_names),
            lowering_input_output_aliases=(),
            sim_require_finite=True,
            sim_require_nnan=True,
            nc=nc,
        )
        return tuple(outs)

    devices = jax.devices()[:n_cores]
    mesh = Mesh(np.asarray(devices), ("core",))
    fn = jax.jit(
        shard_map(
            _body,
            mesh=mesh,
            in_specs=(PartitionSpec("core"),) * (n_params + n_outs),
            out_specs=(PartitionSpec("core"),) * n_outs,
            check_rep=False,
        ),
        keep_unused=True,
    )

    class Runner:
        in_names_ = in_names
        out_names_ = out_names

        def prep(self, in_maps):
            import jax as _jax

            concat_in = [
                np.concatenate(
                    [np.ascontiguousarray(in_maps[c][n]) for c in range(n_cores)],
                    axis=0,
                )
                for n in in_names
            ]
            concat_zero = [
                np.zeros((n_cores * a.shape[0], *a.shape[1:]), a.dtype)
                for a in out_avals
            ]
            shardings = [
                _jax.sharding.NamedSharding(mesh, PartitionSpec("core"))
            ] * (n_params + n_outs)
            return _jax.device_put(concat_in + concat_zero, shardings)

        def run(self, args):
            import jax as _jax

            outs = fn(*args)
            _jax.block_until_ready(outs)
            return [
                {
                    n: np.asarray(outs[i]).reshape(n_cores, *out_avals[i].shape)[c]
                    for i, n in enumerate(out_names)
                }
                for c in range(n_cores)
            ]

        def time_pipelined(self, args, reps=10, warmup=1):
            import time as _time

            import jax as _jax

            for _ in range(warmup):
                _jax.block_until_ready(fn(*args))
            t0 = _time.perf_counter()
            outs = None
            for _ in range(reps):
                outs = fn(*args)
            _jax.block_until_ready(outs)
            return (_time.perf_counter() - t0) / reps

    return Runner()


def _shard_inputs(x, cos, sin, mask, wq, wk, wv, wo, gate, adapter):
    """Build the 8 per-core input maps."""
    import ml_dtypes

    bf = ml_dtypes.bfloat16
    # rope permutation of head-dim columns: even dims first, odd second
    perm = np.concatenate(
        [np.arange(0, HD, 2), np.arange(1, HD, 2)]
    )  # within one head
    col_perm = np.concatenate(
        [h * HD + perm for h in range(H)]
    )  # all heads, head-major
    wq_f = np.asarray(wq, dtype=np.float32)[:, col_perm]
    wk_f = np.asarray(wk, dtype=np.float32)[:, col_perm]
    wv_f = np.asarray(wv, dtype=np.float32)
    wq_p = wq_f.astype(bf)
    wk_p = wk_f.astype(bf)
    wv_b = wv_f.astype(bf)
    wo_b = np.asarray(wo, dtype=np.float32).astype(bf)

    DC = D // P
    HPC_ = HPC
    WC3 = 2 * HPC_  # phase-3 contraction chunks (pair heads)
    ET3 = D // 512 // 2  # phase-3 out tiles (half of D per core)

    def _prearrange_w(w_slice):
        # [D, DH] -> [P, HPC, DC, HD]: contiguous per-partition head tiles
        return np.ascontiguousarray(
            w_slice.reshape(DC, P, HPC_, HD).transpose(1, 2, 0, 3)
        )

    def _prearrange_wo(wo_slice):
        # [2*DH, D/2] (pair-head rows interleaved (h, j)) -> [P, ET3, WC3, 512]
        return np.ascontiguousarray(
            wo_slice.reshape(WC3, P, ET3, 512).transpose(1, 2, 0, 3)
        )

    cosT = np.ascontiguousarray(cos.T, dtype=np.float32)  # [64, S]
    sinT = np.ascontiguousarray(sin.T, dtype=np.float32)

    # 0/1 [k, q] allowed-mask of an aligned 128x128 diagonal block,
    # derived from the mask input (k <= q allowed)
    m = np.asarray(mask, dtype=np.float32)[0, 0]  # [S, S]
    tri = np.ascontiguousarray((m[:P, :P].T == 0)).astype(np.float32)

    gate_v = np.asarray(gate, dtype=np.float32).reshape(H)  # per head
    ad_f = np.asarray(adapter, dtype=np.float32)[0]  # [AL, D]

    xT = [
        np.ascontiguousarray(
            np.asarray(x[b], dtype=np.float32)
            .T.astype(bf)
            .reshape(D // P, P, S)
            .transpose(1, 0, 2)
        )
        for b in range(B)
    ]  # [P, DC, S]

    in_maps = []
    for c in range(N_CORES):
        b = c // TP
        g = c % TP
        mshp = g & 1  # member slot within the pair
        hs = g * HPC * HD  # column slice start

        # host-computed adapter projections for this core's heads
        ak = ad_f @ wk_f[:, hs : hs + HPC * HD]  # [AL, HPC*HD], rope basis
        akT = np.ascontiguousarray(
            ak.reshape(AL, HPC_, HD).transpose(2, 1, 0)
        ).astype(bf)  # [P(hd), HPC, AL]
        av = ad_f @ wv_f[:, hs : hs + HPC * HD]  # [AL, HPC*HD]
        av = av.reshape(AL, HPC_, HD) * gate_v[g * HPC : (g + 1) * HPC][None, :, None]
        av = np.ascontiguousarray(av).astype(bf)  # [AL, HPC, P]

        # phase-3 wo: rows = the PAIR's 16 heads (2*DH), reordered so chunk
        # w = h*2 + j picks (member j, head h); cols = this member's D-half
        g2 = g & ~1
        wo_pair = wo_b[g2 * HPC * HD : (g2 + 2) * HPC * HD,
                       mshp * (D // 2) : (mshp + 1) * (D // 2)]
        wo_hj = np.ascontiguousarray(
            wo_pair.reshape(2, HPC_, HD, D // 2)
            .transpose(1, 0, 2, 3)
            .reshape(2 * HPC_ * HD, D // 2)
        )

        in_maps.append(
            {
                "xT": xT[b],
                "wq": _prearrange_w(wq_p[:, hs : hs + HPC * HD]),
                "wk": _prearrange_w(wk_p[:, hs : hs + HPC * HD]),
                "wv": _prearrange_w(wv_b[:, hs : hs + HPC * HD]),
                "wo": _prearrange_wo(wo_hj),
                "akT": akT,
                "av": av,
                "cosT": cosT,
                "sinT": sinT,
                "tri": tri,
            }
        )
    return in_maps


def get_runner():
    global _RUNNER
    if _RUNNER is None:
        nc = build_nc(Cfg())
        _RUNNER = _make_runner(nc)
    return _RUNNER


def kernel(**inputs) -> np.ndarray:
    x = np.asarray(inputs["x"])
    in_maps = _shard_inputs(
        x,
        inputs["cos"],
        inputs["sin"],
        inputs["mask"],
        inputs["wq"],
        inputs["wk"],
        inputs["wv"],
        inputs["wo"],
        inputs["gate"],
        inputs["adapter"],
    )
    runner = get_runner()
    args = runner.prep(in_maps)
    outs = runner.run(args)
    # each core returns y[:, its D-half] summed over its 8 heads; two cores
    # per (batch, D-half) pair up (head-groups {0,1} and {2,3})
    y = np.zeros((B, S, D), dtype=np.float32)
    for c in range(N_CORES):
        b = c // TP
        mshp = (c % TP) & 1
        y[b][:, mshp * (D // 2) : (mshp + 1) * (D // 2)] += outs[c]["y"]
    return y

